# revision 1
# baseline (speedup 1.0000x reference)
"""Trainium2 Bass kernel for nn_DegreePrediction (RBC via batched Perron vectors).

Math: M[s,t] = weights_r*r_zeros + r_const is positive column-stochastic
(columns sum to 1), so its eigenvalue-1 right eigenvector is the Perron
vector, and the reference formula  rbc[n] = sum_{s,t} T[s,t]/v[s,t,s] * v[s,t,n]
is invariant to the scale of v.  Columns of M^4 converge to v at rate
lambda2^4 (lambda2 <= ~0.09 here), so two batched 64x64 matmul squarings
per (s,t) pair give v to fp32 accuracy.

Sharding: the 4096 (s,t) pairs are split by s across 8 cores (512 pairs
each).  Each core computes a partial 64-vector; the host sums the 8 partials.

Device pipeline per chunk of 8 matrices: all PE transposes are "double
transposes" ([64,128] input -> [128,64] output) at tile_position (0,0) —
transpose-mode matmuls crash the PE when consecutive ops switch row
groups, and the Tile scheduler is free to interleave transposes from
different chunks, so every transpose must use the same quadrant.  The
double transpose also lands the pair 2-stacked across partition halves,
which is exactly the layout the (quadrant-alternating, crash-safe)
squaring matmuls need.  DVE/ACT split the PSUM->SBUF copies (partition-
shifted where the half structure must be rebuilt); the per-pair
denominators v[s] are gathered on DVE with a mask in the transposed-V
layout because PE ones-matmuls with a 128-wide lhsT lose ~11 bits.
"""

import numpy as np

_N = 64
_NCORES = 8
_NP = 512          # pairs per core
_NCHUNK = 64       # chunks of 8 pairs

_cached = {}


def _build_program():
    import concourse.tile as tile
    from concourse import bacc, mybir
    from contextlib import ExitStack

    f32 = mybir.dt.float32
    nc = bacc.Bacc("TRN2", target_bir_lowering=False, debug=False)
    m_in = nc.dram_tensor("m", [_NP, _N, _N], f32, kind="ExternalInput").ap()
    mask_in = nc.dram_tensor("mask", [128, 4, _N], f32, kind="ExternalInput").ap()
    xpp_in = nc.dram_tensor("xpp", [128, 4], f32, kind="ExternalInput").ap()
    wpp_in = nc.dram_tensor("wpp", [128, 4], f32, kind="ExternalInput").ap()
    rpp_in = nc.dram_tensor("rpp", [128, 4], f32, kind="ExternalInput").ap()
    ident_in = nc.dram_tensor("ident", [_N, _N], f32, kind="ExternalInput").ap()
    out_dram = nc.dram_tensor("out", [_N, 1], f32, kind="ExternalOutput").ap()

    with tile.TileContext(nc) as tc:
        with ExitStack() as ctx:
            consts = ctx.enter_context(tc.tile_pool(name="consts", bufs=1))
            work = ctx.enter_context(tc.tile_pool(name="work", bufs=4))
            psum = ctx.enter_context(tc.tile_pool(name="psum", bufs=2, space="PSUM"))

            ident = consts.tile([_N, _N], f32)
            nc.sync.dma_start(out=ident[:, :], in_=ident_in[:, :])
            mask_sb = consts.tile([128, 4, _N], f32)
            nc.sync.dma_start(out=mask_sb[:, :, :], in_=mask_in[:, :, :])
            xpp_sb = consts.tile([128, 4], f32)
            nc.sync.dma_start(out=xpp_sb[:, :], in_=xpp_in[:, :])
            wpp_sb = consts.tile([128, 4], f32)
            nc.sync.dma_start(out=wpp_sb[:, :], in_=wpp_in[:, :])
            rpp_sb = consts.tile([128, 4], f32)
            nc.sync.dma_start(out=rpp_sb[:, :], in_=rpp_in[:, :])
            ones = consts.tile([128, 1], f32)
            nc.vector.memset(ones[:, :], 1.0)
            v_sb = consts.tile([128, _NCHUNK, 4], f32)

            def split2(t):
                """[64|128, 8, 64] tile -> (even-slot view, odd-slot view)."""
                r = t[:, :, :].rearrange("p (c two) j -> p c two j", two=2)
                return r[:, :, 0, :], r[:, :, 1, :]

            for k in range(_NCHUNK):
                mc64 = work.tile([_N, 8, _N], f32, tag="mc64")
                nc.sync.dma_start(
                    out=mc64[:, :, :],
                    in_=m_in[8 * k: 8 * k + 8, :, :].rearrange("p i j -> i p j"))
                # 2-stacked copy for matmul rhs: mcS[64h+i, d] = M_{8k+2d+h}
                mcS = work.tile([128, 4, _N], f32, tag="mcS")
                mc_ev, mc_od = split2(mc64)
                nc.scalar.copy(out=mcS[0:64, :, :], in_=mc_ev)
                nc.vector.tensor_copy(out=mcS[64:128, :, :], in_=mc_od)
                # double transposes: pt[:, d] = [M_{2d}^T ; M_{2d+1}^T] stacked
                pt = psum.tile([128, 4, _N], f32, tag="pt")
                for d in range(4):
                    nc.tensor.transpose(
                        out=pt[:, d, :],
                        in_=mc64[:, 2 * d:2 * d + 2, :].rearrange("p a j -> p (a j)"),
                        identity=ident[:, :])
                mtS = work.tile([128, 4, _N], f32, tag="mtS")
                nc.vector.tensor_copy(out=mtS[:, :, :], in_=pt[:, :, :])
                # M^2, quadrant-alternating (safe for regular matmuls)
                pp = psum.tile([128, 4, _N], f32, tag="pp")
                for d in range(4):
                    for h in (0, 1):
                        b = 64 * h
                        nc.tensor.matmul(
                            out=pp[b:b + 64, d, :],
                            lhsT=mtS[b:b + 64, d, :],
                            rhs=mcS[b:b + 64, d, :],
                            start=True, stop=True)
                p1S = work.tile([128, 4, _N], f32, tag="p1S")
                nc.scalar.copy(out=p1S[:, :, :], in_=pp[:, :, :])
                p1_64 = work.tile([_N, 8, _N], f32, tag="p1_64")
                p1_ev, p1_od = split2(p1_64)
                nc.scalar.copy(out=p1_ev, in_=pp[0:64, :, :])
                nc.vector.tensor_copy(out=p1_od, in_=pp[64:128, :, :])
                pt2 = psum.tile([128, 4, _N], f32, tag="pt2")
                for d in range(4):
                    nc.tensor.transpose(
                        out=pt2[:, d, :],
                        in_=p1_64[:, 2 * d:2 * d + 2, :].rearrange("p a j -> p (a j)"),
                        identity=ident[:, :])
                q1S = work.tile([128, 4, _N], f32, tag="q1S")
                nc.vector.tensor_copy(out=q1S[:, :, :], in_=pt2[:, :, :])
                pp2 = psum.tile([128, 4, _N], f32, tag="pp2")
                for d in range(4):
                    for h in (0, 1):
                        b = 64 * h
                        nc.tensor.matmul(
                            out=pp2[b:b + 64, d, :],
                            lhsT=q1S[b:b + 64, d, :],
                            rhs=p1S[b:b + 64, d, :],
                            start=True, stop=True)
                nc.vector.tensor_reduce(
                    out=v_sb[:, k, :], in_=pp2[:, :, :],
                    axis=mybir.AxisListType.X, op=mybir.AluOpType.add)

            # ---- tail ----
            v_flat = v_sb[:, :, :].rearrange("p a b -> p (a b)")  # [128, 256]
            # V rows to partitions 0-63, then 4 (0,0) double-transposes
            v64 = consts.tile([_N, 2, 256], f32)
            nc.gpsimd.tensor_copy(out=v64[:, 0, :], in_=v_flat[0:64, :])
            nc.gpsimd.tensor_copy(out=v64[:, 1, :], in_=v_flat[64:128, :])
            pvt = psum.tile([128, 4, _N], f32, tag="pp")
            for h in (0, 1):
                for g in (0, 1):
                    j = 2 * h + g
                    nc.tensor.transpose(
                        out=pvt[:, j, :],
                        in_=v64[:, h, 128 * g:128 * g + 128],
                        identity=ident[:, :])
            vt = consts.tile([128, 4, _N], f32)
            nc.vector.tensor_copy(out=vt[:, :, :], in_=pvt[:, :, :])
            # denominators v[s] via VT-layout mask gather on DVE (exact fp32;
            # a PE ones-matmul with 128-wide lhsT loses ~11 bits)
            maskv = consts.tile([128, 4, _N], f32)
            nc.vector.tensor_mul(out=maskv[:, :, :], in0=vt[:, :, :],
                                 in1=mask_sb[:, :, :])
            d_sb = consts.tile([128, 4], f32)
            nc.vector.tensor_reduce(
                out=d_sb[:, :], in_=maskv[:, :, :],
                axis=mybir.AxisListType.X, op=mybir.AluOpType.add)
            dinv = consts.tile([128, 4], f32)
            nc.vector.reciprocal(out=dinv[:, :], in_=d_sb[:, :])
            tpp = consts.tile([128, 4], f32)
            nc.vector.tensor_mul(out=tpp[:, :], in0=xpp_sb[:, :], in1=wpp_sb[:, :])
            nc.vector.tensor_mul(out=tpp[:, :], in0=tpp[:, :], in1=rpp_sb[:, :])
            u = consts.tile([128, 4], f32)
            nc.vector.tensor_mul(out=u[:, :], in0=tpp[:, :], in1=dinv[:, :])
            prbc = psum.tile([_N, 1], f32, tag="pt2")
            for j in range(4):
                nc.tensor.matmul(
                    out=prbc[:, :], lhsT=vt[:, j, :], rhs=u[:, j:j + 1],
                    start=(j == 0), stop=(j == 3))
            out_sb = consts.tile([_N, 1], f32)
            nc.vector.tensor_copy(out=out_sb[:, :], in_=prbc[:, :])
            nc.sync.dma_start(out=out_dram[:, :], in_=out_sb[:, :])
    nc.compile()
    return nc


def _get_program():
    if "nc" not in _cached:
        _cached["nc"] = _build_program()
    return _cached["nc"]


def _pair_of(h, f):
    """Local pair id for half h, V-free-index f (f = 4*chunk + dslot)."""
    return 8 * (f >> 2) + 2 * (f & 3) + h


def _host_layouts(x, weights_t, r_const):
    """Per-core gathers: xpp/wpp/rpp [128,4] pairs-on-partitions, mask [128,256]."""
    Q = np.arange(128)[:, None]
    J = np.arange(4)[None, :]
    h = J >> 1
    g = J & 1
    f = 128 * g + Q
    p = _pair_of(h, f)                      # local pair id [128, 4]
    s_loc = p >> 6
    t = p & 63
    F = np.arange(256)
    outs = []
    for c in range(_NCORES):
        s_glob = 8 * c + s_loc
        xpp = np.ascontiguousarray(x[s_glob, t], np.float32)
        wpp = np.ascontiguousarray(weights_t[s_glob, t], np.float32)
        rpp = np.ascontiguousarray(r_const[s_glob, t, s_glob, s_glob], np.float32)
        # maskT[q, j, i] = 1 iff i == s_glob(pair at VT position (q, j))
        mask = np.zeros((128, 4, _N), np.float32)
        for j in range(4):
            hh = j >> 1
            ff = 128 * (j & 1) + np.arange(128)
            pl = _pair_of(hh, ff)
            sg = 8 * c + (pl >> 6)
            mask[np.arange(128), j, sg] = 1.0
        outs.append((xpp, wpp, rpp, mask))
    return outs


def kernel(x, weights_t, weights_r, r_zeros, r_const):
    from concourse.bass_utils import run_bass_kernel_spmd

    x = np.asarray(x, np.float32)
    weights_t = np.asarray(weights_t, np.float32)
    r_const = np.asarray(r_const, np.float32)
    r_zeros_np = np.asarray(r_zeros)
    if np.any(r_zeros_np):
        M_all = (np.asarray(weights_r, np.float32) * r_zeros_np.astype(np.float32)
                 + r_const).reshape(_N * _N, _N, _N)
    else:
        M_all = r_const.reshape(_N * _N, _N, _N)

    nc = _get_program()
    ident_np = np.eye(_N, dtype=np.float32)
    layouts = _host_layouts(x, weights_t, r_const)
    in_maps = []
    for c in range(_NCORES):
        xpp, wpp, rpp, mask = layouts[c]
        in_maps.append({
            "m": np.ascontiguousarray(M_all[_NP * c:_NP * (c + 1)], np.float32),
            "mask": mask,
            "xpp": xpp,
            "wpp": wpp,
            "rpp": rpp,
            "ident": ident_np,
        })
    res = run_bass_kernel_spmd(nc, in_maps, core_ids=list(range(_NCORES)))
    parts = np.stack([r["out"][:, 0] for r in res.results])  # [8, 64]
    return parts.sum(axis=0, dtype=np.float64).astype(np.float32)

